# revision 35
# baseline (speedup 1.0000x reference)
"""Causal single-head attention (B=4, T=4096, D=1024) on 8 trn2 NeuronCores.

Sharding: 2 cores per batch element, split by key-block PARITY (flash-style):
  core = 2*b + p ; p in {0,1}
  Each core computes, for ALL 4096 queries of batch b, the partial
  (unnormalized) attention output over its 16 key blocks {128*(2u+p)} and the
  partial softmax row-sums. Host merges: O = (O_0 + O_1) / (rs_0 + rs_1).
  exp() without per-row max subtraction (scaled scores are in [-8, 8] for
  randn inputs; exp stays well inside fp32 range).

v4: q-projection split between the two cores of a pair: core p computes qT
  for 4 of the 8 query chunk-pairs (host-prepared xqo input), exchanged via
  4 small pairwise AllGathers fired as each 512-col slot completes, ordered
  so the earliest-needed pairs exchange first. Attention chunks run in order
  [11..0, 15, 14, 13, 12]: gathered qT is first needed well after AllGather
  #0 completes, and the kernel ends on big chunks that hide output drains.
  qT reads ride the gpsimd queue (they depend on the collectives, which must
  stay on gpsimd); output drains ride sync so they never queue behind a
  collective. All matmul operands bf16 (FWL); PSUM accumulation fp32.

  slot s of core p holds query chunk-pair SLOT_PAIRS[s] + p; qall_s gathers
  the even core's slot (rank 0) then the odd core's (rank 1).
"""

import sys

sys.path.insert(0, "/opt/trn_rl_repo")

import numpy as np
import ml_dtypes
from contextlib import ExitStack

import concourse.tile as tile
from concourse import bacc, mybir
from concourse.bass_utils import run_bass_kernel_spmd

P = 128
D = 1024
T = 4096
B = 4
NDB = D // P  # 8 d-blocks
NCB = D // P  # 8 contraction blocks
NKB = 16  # key blocks per core (parity half of 32)
QC = 256  # query-chunk columns in phase C
NQC = T // QC  # 16
CH = 512  # streaming column chunk
NSLOT = 4  # q chunk-pairs computed locally per core
SLOT_PAIRS = [4, 2, 0, 6]  # + core parity; exchange order = first-needed first
SLOT_OF_PAIR = {4: 0, 5: 0, 2: 1, 3: 1, 0: 2, 1: 2, 6: 3, 7: 3}
F32 = mybir.dt.float32
BF16 = mybir.dt.bfloat16
EXPSCALE = 1.0 / 32.0  # 1/sqrt(D)
EXP = mybir.ActivationFunctionType.Exp
GROUPS = [[0, 1], [2, 3], [4, 5], [6, 7]]
# Small chunks (0-3) are sandwiched between big ones so every drain lands at
# the start of a chunk with enough score matmuls to hide the copy latency;
# ends on the biggest chunks so output drains overlap their streams.
ORDER = [11, 10, 2, 9, 1, 8, 0, 7, 3, 6, 5, 4, 15, 14, 13, 12]

_CACHED_NC = None
_LAST_RES = None


def _build_program():
    nc = bacc.Bacc("TRN2", target_bir_lowering=False, debug=False, num_devices=8)

    xqo_d = nc.dram_tensor("xqo", [D, NSLOT * CH], BF16, kind="ExternalInput").ap()
    xTk_d = nc.dram_tensor("xTk", [D, T // 2], BF16, kind="ExternalInput").ap()
    wq_d = nc.dram_tensor("WqT", [D, D], BF16, kind="ExternalInput").ap()
    wk_d = nc.dram_tensor("WkT", [D, D], BF16, kind="ExternalInput").ap()
    wv_d = nc.dram_tensor("WvT", [D, D], BF16, kind="ExternalInput").ap()
    mask_d = nc.dram_tensor("mask", [P, QC], F32, kind="ExternalInput").ap()
    ones2_d = nc.dram_tensor("ones2", [P, NKB * 2], BF16, kind="ExternalInput").ap()
    o_d = nc.dram_tensor("O", [T, D], BF16, kind="ExternalOutput").ap()
    rs_d = nc.dram_tensor("rs", [T, 1], F32, kind="ExternalOutput").ap()
    qown = [nc.dram_tensor(f"qown{s}", [D, CH], BF16).ap() for s in range(NSLOT)]
    qall = [nc.dram_tensor(f"qall{s}", [2 * D, CH], BF16).ap() for s in range(NSLOT)]

    xqo_r = xqo_d.rearrange("(a p) c -> p a c", p=P)  # [128, 8, 2048]
    xTk_r = xTk_d.rearrange("(a p) t -> p a t", p=P)  # [128, 8, 2048]
    wq_r = wq_d.rearrange("(a p) d -> p a d", p=P)  # [128, 8, 1024]
    wk_r = wk_d.rearrange("(a p) d -> p a d", p=P)
    wv_r = wv_d.rearrange("(a p) d -> p a d", p=P)
    qown_r = [q.rearrange("(a p) c -> p a c", p=P) for q in qown]  # [128, 8, 512]
    qall_r = [
        q.rearrange("(r a p) c -> p (r a) c", p=P, r=2) for q in qall
    ]  # [128, 16, 512]

    with tile.TileContext(nc) as tc, ExitStack() as ctx:
        res = ctx.enter_context(tc.tile_pool(name="res", bufs=1))
        xkp = ctx.enter_context(tc.tile_pool(name="xkp", bufs=2))
        xqp = ctx.enter_context(tc.tile_pool(name="xqp", bufs=3))
        qop = ctx.enter_context(tc.tile_pool(name="qop", bufs=2))
        qtp = ctx.enter_context(tc.tile_pool(name="qtp", bufs=4))
        pp = ctx.enter_context(tc.tile_pool(name="pp", bufs=4))
        stg = ctx.enter_context(tc.tile_pool(name="stg", bufs=6))
        psum = ctx.enter_context(tc.tile_pool(name="psum", bufs=1, space="PSUM"))

        wq_res = res.tile([P, NCB, D], BF16, tag="wq")
        wk_res = res.tile([P, NCB, D], BF16, tag="wk")
        wv_res = res.tile([P, NCB, D], BF16, tag="wv")
        kt_t = res.tile([P, NDB, T // 2], BF16, tag="kt")  # [128, 8, 2048]
        v_t = res.tile([P, NKB, D + 2], BF16, tag="vt")  # [128, 16, 1026]
        mask_t = res.tile([P, QC], F32, tag="mask")
        rs_all = res.tile([P, 2 * NQC], F32, tag="rs")  # row-sums, one col per
        # (chunk, sub); single strided DMA to rs_d at the end

        # ---- head DMAs, ordered for the first q-projection matmuls ----------
        nc.sync.dma_start(wq_res[:, :, 0 : 2 * P], wq_r[:, :, 0 : 2 * P])
        xqs = {}

        def xq_dma(s):
            xqs[s] = xqp.tile([P, NCB, CH], BF16, tag="xq", name=f"xq{s}")
            nc.sync.dma_start(xqs[s][:], xqo_r[:, :, s * CH : (s + 1) * CH])

        xq_dma(0)
        nc.sync.dma_start(wq_res[:, :, 2 * P : 4 * P], wq_r[:, :, 2 * P : 4 * P])
        xq_dma(1)
        nc.sync.dma_start(wq_res[:, :, 4 * P : 6 * P], wq_r[:, :, 4 * P : 6 * P])
        xq_dma(2)
        nc.sync.dma_start(wq_res[:, :, 6 * P : D], wq_r[:, :, 6 * P : D])
        nc.sync.dma_start(wk_res[:, :, 0:P], wk_r[:, :, 0:P])
        nc.gpsimd.dma_start(mask_t[:], mask_d[:])
        ones2_r = ones2_d.rearrange("p (k two) -> p k two", two=2)
        nc.gpsimd.dma_start(v_t[:, :, D : D + 2], ones2_r[:])

        # ---------------- Phase A': local q-projection + exchange ------------
        for s in range(NSLOT):
            xq = xqs.pop(s)
            qo = qop.tile([P, NDB, CH], BF16, tag="qo", name=f"qo{s}")
            for db in range(NDB):
                ps = psum.tile([P, CH], F32, tag=f"b{6 + db % 2}", name=f"qp{s}_{db}")
                for cb in range(NCB):
                    nc.tensor.matmul(
                        ps[:],
                        wq_res[:, cb, db * P : (db + 1) * P],
                        xq[:, cb, :],
                        start=(cb == 0),
                        stop=(cb == NCB - 1),
                    )
                nc.vector.tensor_copy(qo[:, db, :], ps[:])
            nc.scalar.dma_start(qown_r[s][:], qo[:])
            if s == 0:  # slot 3's input, prefetched once slot 0's is free
                xq_dma(3)
            nc.gpsimd.collective_compute(
                "AllGather",
                mybir.AluOpType.bypass,
                replica_groups=GROUPS,
                ins=[qown[s][:]],
                outs=[qall[s][:]],
            )

        # ---------------- Phase B: kT + V (resident, bf16) -------------------
        xks = {}

        def xk_dma(g):
            xks[g] = xkp.tile([P, NCB, CH], BF16, tag="xk", name=f"xk{g}")
            nc.sync.dma_start(xks[g][:], xTk_r[:, :, g * CH : (g + 1) * CH])

        xk_dma(0)
        nc.sync.dma_start(wk_res[:, :, P:D], wk_r[:, :, P:D])
        nc.sync.dma_start(wv_res[:], wv_r[:])
        for g in range(4):  # groups of 4 key blocks (512 cols of xTk)
            if g + 1 < 4:
                xk_dma(g + 1)
            xk = xks.pop(g)
            for db in range(NDB):
                ps = psum.tile([P, CH], F32, tag=f"b{6 + db % 2}")
                for cb in range(NCB):
                    nc.tensor.matmul(
                        ps[:],
                        wk_res[:, cb, db * P : (db + 1) * P],
                        xk[:, cb, :],
                        start=(cb == 0),
                        stop=(cb == NCB - 1),
                    )
                nc.vector.tensor_copy(kt_t[:, db, g * CH : (g + 1) * CH], ps[:])
            for i in range(4):
                kb = 4 * g + i
                for h in range(2):
                    ps = psum.tile([P, CH], F32, tag=f"b{(2 * i + h) % 4}")
                    for cb in range(NCB):
                        nc.tensor.matmul(
                            ps[:],
                            xk[:, cb, i * P : (i + 1) * P],
                            wv_res[:, cb, h * CH : (h + 1) * CH],
                            start=(cb == 0),
                            stop=(cb == NCB - 1),
                        )
                    nc.vector.tensor_copy(v_t[:, kb, h * CH : (h + 1) * CH], ps[:])

        # ---------------- Phase C: attention (software-pipelined) -------------
        qt_tiles = {}

        def qt_dma(c):
            k = c // 2
            s = SLOT_OF_PAIR[k]
            t = qtp.tile([P, NDB, QC], BF16, tag="qt", name=f"qt{c}")
            col = (c % 2) * QC
            nc.gpsimd.dma_start(
                t[:], qall_r[s][:, (k % 2) * NDB : (k % 2 + 1) * NDB, col : col + QC]
            )
            qt_tiles[c] = t

        for c in ORDER[:3]:
            qt_dma(c)

        prev = None  # (acc dict, j) pending drain
        for oi, j in enumerate(ORDER):
            if oi + 3 < NQC:
                qt_dma(ORDER[oi + 3])
            qt = qt_tiles.pop(j)
            acc = {}
            for sub in range(2):
                for c in range(3):
                    shape = [P, 2] if c == 2 else [P, 512]
                    acc[sub, c] = psum.tile(
                        shape, F32, tag=f"b{sub * 3 + c}", name=f"acc{j}_{sub}_{c}"
                    )

            def av(u, pt_t, first, last):
                for sub in range(2):
                    lhs = pt_t[:, sub * P : (sub + 1) * P]
                    nc.tensor.matmul(
                        acc[sub, 0][:], lhs, v_t[:, u, 0:512],
                        start=first, stop=last, skip_group_check=True,
                    )
                    nc.tensor.matmul(
                        acc[sub, 1][:], lhs, v_t[:, u, 512:1024],
                        start=first, stop=last, skip_group_check=True,
                    )
                    nc.tensor.matmul(
                        acc[sub, 2][:], lhs, v_t[:, u, D : D + 2],
                        start=first, stop=last, skip_group_check=True,
                    )

            def drain(d_acc, d_j):
                # All big copies on vector, in the order the next chunk's AV
                # matmuls will reuse the banks (b0, b3, b1, b4); exps stay
                # alone on scalar since they gate the AV weight loads. Row
                # sums land in the resident rs_all tile (no DMA until the
                # very end).
                ot = {}
                for c, sub in [(0, 0), (0, 1), (1, 0), (1, 1)]:
                    t = stg.tile([P, 512], BF16, tag="stage", name=f"ot{c}_{d_j}_{sub}")
                    nc.vector.tensor_copy(t[:], d_acc[sub, c][:])
                    ot[sub, c] = t
                for sub in range(2):
                    row = d_j * QC + sub * P
                    col = 2 * d_j + sub
                    nc.scalar.copy(rs_all[:, col : col + 1], d_acc[sub, 2][:, 0:1])
                    nc.sync.dma_start(o_d[row : row + P, 0:512], ot[sub, 0][:])
                    nc.sync.dma_start(o_d[row : row + P, 512:1024], ot[sub, 1][:])

            if prev is not None:  # drain at chunk start: copies overlap scores
                drain(*prev)
                prev = None

            pts = {}
            navs = [0]

            def do_av(u, last):
                av(u, pts.pop(u), first=(navs[0] == 0), last=last)
                navs[0] += 1

            for u in range(j + 1):
                st = psum.tile([P, QC], F32, tag=f"b{6 + u % 2}", name=f"st{j}_{u}")
                for db in range(NDB):
                    nc.tensor.matmul(
                        st[:],
                        kt_t[:, db, u * P : (u + 1) * P],
                        qt[:, db, :],
                        start=(db == 0),
                        stop=(db == NDB - 1),
                    )
                if u == j:
                    nc.vector.tensor_add(st[:], st[:], mask_t[:])
                pt = pp.tile([P, QC], BF16, tag="pt", name=f"pt{j}_{u}")
                nc.scalar.activation(pt[:], st[:], EXP, scale=EXPSCALE)
                pts[u] = pt
                if u >= 3:  # AV lag 3: drain copies + exps get a 3-block lead
                    do_av(u - 3, last=False)
            for u in range(max(0, j - 2), j + 1):
                do_av(u, last=(u == j))
            prev = (acc, j)
        drain_acc, drain_j = prev
        for sub in range(2):
            row = drain_j * QC + sub * P
            col = 2 * drain_j + sub
            ot0 = stg.tile([P, 512], BF16, tag="stage", name=f"fot0_{sub}")
            nc.vector.tensor_copy(ot0[:], drain_acc[sub, 0][:])
            ot1 = stg.tile([P, 512], BF16, tag="stage", name=f"fot1_{sub}")
            nc.scalar.copy(ot1[:], drain_acc[sub, 1][:])
            nc.scalar.copy(rs_all[:, col : col + 1], drain_acc[sub, 2][:, 0:1])
            nc.sync.dma_start(o_d[row : row + P, 0:512], ot0[:])
            nc.sync.dma_start(o_d[row : row + P, 512:1024], ot1[:])
        rs_r = rs_d.rearrange("(c p) one -> p (c one)", p=P)  # [128, 32]
        nc.scalar.dma_start(rs_r[:], rs_all[:])

    nc.finalize()
    return nc


def _get_program():
    global _CACHED_NC
    if _CACHED_NC is None:
        _CACHED_NC = _build_program()
    return _CACHED_NC


def _masks():
    neg = np.float32(-1e30)
    tri = np.where(np.triu(np.ones((P, P), dtype=bool)), np.float32(0), neg)
    keep = np.zeros((P, P), dtype=np.float32)
    drop = np.full((P, P), neg, dtype=np.float32)
    return (
        np.ascontiguousarray(np.concatenate([tri, keep], axis=1)),  # even core
        np.ascontiguousarray(np.concatenate([drop, tri], axis=1)),  # odd core
    )


def kernel(x, Wq, Wk, Wv):
    out, _ = _run(x, Wq, Wk, Wv, trace=False)
    return out


def _run(x, Wq, Wk, Wv, trace=False, keep_res=False):
    bf = ml_dtypes.bfloat16
    x = np.asarray(x, dtype=np.float32)
    WqT = np.ascontiguousarray(np.asarray(Wq, dtype=np.float32).T.astype(bf))
    WkT = np.ascontiguousarray(np.asarray(Wk, dtype=np.float32).T.astype(bf))
    WvT = np.ascontiguousarray(np.asarray(Wv, dtype=np.float32).T.astype(bf))
    m_even, m_odd = _masks()
    ones2 = np.ascontiguousarray(
        np.tile(np.array([[1.0, 0.0] * NKB], dtype=np.float32), (P, 1)).astype(bf)
    )

    nc = _get_program()
    in_maps = []
    for core in range(8):
        b, p = core // 2, core % 2
        xT = np.ascontiguousarray(x[b].T.astype(bf))  # [D, T]
        xTk = np.ascontiguousarray(
            xT.reshape(D, T // P, P)[:, p::2, :].reshape(D, T // 2)
        )
        xqo = np.ascontiguousarray(
            np.concatenate(
                [
                    xT[:, CH * (k + p) : CH * (k + p + 1)]
                    for k in SLOT_PAIRS
                ],
                axis=1,
            )
        )
        in_maps.append(
            {
                "xqo": xqo,
                "xTk": xTk,
                "WqT": WqT,
                "WkT": WkT,
                "WvT": WvT,
                "mask": m_even if p == 0 else m_odd,
                "ones2": ones2,
            }
        )

    res = run_bass_kernel_spmd(nc, in_maps, core_ids=list(range(8)), trace=trace)
    if keep_res:
        global _LAST_RES
        _LAST_RES = res
    out = np.empty((B, T, D), dtype=np.float32)
    for b in range(B):
        O0 = res.results[2 * b]["O"].astype(np.float32)
        rs0 = res.results[2 * b]["rs"]
        O1 = res.results[2 * b + 1]["O"].astype(np.float32)
        rs1 = res.results[2 * b + 1]["rs"]
        out[b] = (O0 + O1) / (rs0 + rs1)
    return out, res.exec_time_ns


# revision 40
# speedup vs baseline: 1.0666x; 1.0666x over previous
"""Causal single-head attention (B=4, T=4096, D=1024) on 8 trn2 NeuronCores.

Sharding: 2 cores per batch element, split by key-block PARITY (flash-style):
  core = 2*b + p ; p in {0,1}
  Each core computes, for ALL 4096 queries of batch b, the partial
  (unnormalized) attention output over its 16 key blocks {128*(2u+p)} and the
  partial softmax row-sums. Host merges: O = (O_0 + O_1) / (rs_0 + rs_1).
  exp() without per-row max subtraction (scaled scores are in [-8, 8] for
  randn inputs; exp stays well inside fp32 range).

v4: q-projection split between the two cores of a pair: core p computes qT
  for 4 of the 8 query chunk-pairs (host-prepared xqo input), exchanged via
  4 small pairwise AllGathers fired as each 512-col slot completes, ordered
  so the earliest-needed pairs exchange first. Attention chunks run in order
  [11..0, 15, 14, 13, 12]: gathered qT is first needed well after AllGather
  #0 completes, and the kernel ends on big chunks that hide output drains.
  qT reads ride the gpsimd queue (they depend on the collectives, which must
  stay on gpsimd); output drains ride sync so they never queue behind a
  collective. All matmul operands bf16 (FWL); PSUM accumulation fp32.

  slot s of core p holds query chunk-pair SLOT_PAIRS[s] + p; qall_s gathers
  the even core's slot (rank 0) then the odd core's (rank 1).
"""

import sys

sys.path.insert(0, "/opt/trn_rl_repo")

import numpy as np
import ml_dtypes
from contextlib import ExitStack

import concourse.tile as tile
from concourse import bacc, mybir
from concourse.bass_utils import run_bass_kernel_spmd

P = 128
D = 1024
T = 4096
B = 4
NDB = D // P  # 8 d-blocks
NCB = D // P  # 8 contraction blocks
NKB = 16  # key blocks per core (parity half of 32)
QC = 256  # query-chunk columns in phase C
NQC = T // QC  # 16
CH = 512  # streaming column chunk
NSLOT = 4  # q chunk-pairs computed locally per core
SLOT_PAIRS = [4, 2, 0, 6]  # + core parity; exchange order = first-needed first
SLOT_OF_PAIR = {4: 0, 5: 0, 2: 1, 3: 1, 0: 2, 1: 2, 6: 3, 7: 3}
F32 = mybir.dt.float32
BF16 = mybir.dt.bfloat16
EXPSCALE = 1.0 / 32.0  # 1/sqrt(D)
EXP = mybir.ActivationFunctionType.Exp
GROUPS = [[0, 1], [2, 3], [4, 5], [6, 7]]
# Small chunks (0-3) are sandwiched between big ones so every drain lands at
# the start of a chunk with enough score matmuls to hide the copy latency;
# ends on the biggest chunks so output drains overlap their streams.
ORDER = [11, 10, 2, 9, 1, 8, 0, 7, 3, 6, 5, 4, 15, 14, 13, 12]

_CACHED_NC = None
_LAST_RES = None


def _build_program():
    nc = bacc.Bacc("TRN2", target_bir_lowering=False, debug=False, num_devices=8)

    xqo_d = nc.dram_tensor("xqo", [D, NSLOT * CH], BF16, kind="ExternalInput").ap()
    xTk_d = nc.dram_tensor("xTk", [D, T // 2], BF16, kind="ExternalInput").ap()
    wq_d = nc.dram_tensor("WqT", [D, D], BF16, kind="ExternalInput").ap()
    wk_d = nc.dram_tensor("WkT", [D, D], BF16, kind="ExternalInput").ap()
    wv_d = nc.dram_tensor("WvT", [D, D], BF16, kind="ExternalInput").ap()
    mask_d = nc.dram_tensor("mask", [P, QC], F32, kind="ExternalInput").ap()
    ones2_d = nc.dram_tensor("ones2", [P, NKB * 2], BF16, kind="ExternalInput").ap()
    o_d = nc.dram_tensor("O", [T, D], BF16, kind="ExternalOutput").ap()
    # row-sum for DRAM row r of O lives at rs[r % 128, r // 128] (host reorders)
    rs_d = nc.dram_tensor("rs", [P, 2 * NQC], F32, kind="ExternalOutput").ap()
    qown = [nc.dram_tensor(f"qown{s}", [D, CH], BF16).ap() for s in range(NSLOT)]
    qall = [nc.dram_tensor(f"qall{s}", [2 * D, CH], BF16).ap() for s in range(NSLOT)]

    xqo_r = xqo_d.rearrange("(a p) c -> p a c", p=P)  # [128, 8, 2048]
    xTk_r = xTk_d.rearrange("(a p) t -> p a t", p=P)  # [128, 8, 2048]
    wq_r = wq_d.rearrange("(a p) d -> p a d", p=P)  # [128, 8, 1024]
    wk_r = wk_d.rearrange("(a p) d -> p a d", p=P)
    wv_r = wv_d.rearrange("(a p) d -> p a d", p=P)
    qown_r = [q.rearrange("(a p) c -> p a c", p=P) for q in qown]  # [128, 8, 512]
    qall_r = [
        q.rearrange("(r a p) c -> p (r a) c", p=P, r=2) for q in qall
    ]  # [128, 16, 512]

    with tile.TileContext(nc) as tc, ExitStack() as ctx:
        res = ctx.enter_context(tc.tile_pool(name="res", bufs=1))
        xkp = ctx.enter_context(tc.tile_pool(name="xkp", bufs=2))
        xqp = ctx.enter_context(tc.tile_pool(name="xqp", bufs=3))
        qop = ctx.enter_context(tc.tile_pool(name="qop", bufs=2))
        qtp = ctx.enter_context(tc.tile_pool(name="qtp", bufs=4))
        pp = ctx.enter_context(tc.tile_pool(name="pp", bufs=4))
        stg = ctx.enter_context(tc.tile_pool(name="stg", bufs=4))
        psum = ctx.enter_context(tc.tile_pool(name="psum", bufs=1, space="PSUM"))

        wq_res = res.tile([P, NCB, D], BF16, tag="wq")
        wk_res = res.tile([P, NCB, D], BF16, tag="wk")
        wv_res = res.tile([P, NCB, D], BF16, tag="wv")
        kt_t = res.tile([P, NDB, T // 2], BF16, tag="kt")  # [128, 8, 2048]
        v_t = res.tile([P, NKB, D + 2], BF16, tag="vt")  # [128, 16, 1026]
        mask_t = res.tile([P, QC], F32, tag="mask")
        rs_all = res.tile([P, 2 * NQC], F32, tag="rs")  # row-sums, one col per
        # (chunk, sub); single strided DMA to rs_d at the end

        # ---- head DMAs, ordered for the first q-projection matmuls ----------
        nc.sync.dma_start(wq_res[:, :, 0 : 2 * P], wq_r[:, :, 0 : 2 * P])
        xqs = {}

        def xq_dma(s):
            xqs[s] = xqp.tile([P, NCB, CH], BF16, tag="xq", name=f"xq{s}")
            nc.sync.dma_start(xqs[s][:], xqo_r[:, :, s * CH : (s + 1) * CH])

        xq_dma(0)
        nc.sync.dma_start(wq_res[:, :, 2 * P : 4 * P], wq_r[:, :, 2 * P : 4 * P])
        xq_dma(1)
        nc.sync.dma_start(wq_res[:, :, 4 * P : 6 * P], wq_r[:, :, 4 * P : 6 * P])
        xq_dma(2)
        nc.sync.dma_start(wq_res[:, :, 6 * P : D], wq_r[:, :, 6 * P : D])
        nc.sync.dma_start(wk_res[:, :, 0:P], wk_r[:, :, 0:P])
        nc.gpsimd.dma_start(mask_t[:], mask_d[:])
        ones2_r = ones2_d.rearrange("p (k two) -> p k two", two=2)
        nc.gpsimd.dma_start(v_t[:, :, D : D + 2], ones2_r[:])

        # ---------------- Phase A': local q-projection + exchange ------------
        for s in range(NSLOT):
            xq = xqs.pop(s)
            qo = qop.tile([P, NDB, CH], BF16, tag="qo", name=f"qo{s}")
            for db in range(NDB):
                ps = psum.tile([P, CH], F32, tag=f"b{6 + db % 2}", name=f"qp{s}_{db}")
                for cb in range(NCB):
                    nc.tensor.matmul(
                        ps[:],
                        wq_res[:, cb, db * P : (db + 1) * P],
                        xq[:, cb, :],
                        start=(cb == 0),
                        stop=(cb == NCB - 1),
                    )
                nc.vector.tensor_copy(qo[:, db, :], ps[:])
            nc.scalar.dma_start(qown_r[s][:], qo[:])
            if s == 0:  # slot 3's input, prefetched once slot 0's is free
                xq_dma(3)
            nc.gpsimd.collective_compute(
                "AllGather",
                mybir.AluOpType.bypass,
                replica_groups=GROUPS,
                ins=[qown[s][:]],
                outs=[qall[s][:]],
            )

        # ---------------- Phase B: kT + V (resident, bf16) -------------------
        xks = {}

        def xk_dma(g):
            xks[g] = xkp.tile([P, NCB, CH], BF16, tag="xk", name=f"xk{g}")
            nc.sync.dma_start(xks[g][:], xTk_r[:, :, g * CH : (g + 1) * CH])

        xk_dma(0)
        nc.sync.dma_start(wk_res[:, :, P:D], wk_r[:, :, P:D])
        nc.sync.dma_start(wv_res[:], wv_r[:])
        for g in range(4):  # groups of 4 key blocks (512 cols of xTk)
            if g + 1 < 4:
                xk_dma(g + 1)
            xk = xks.pop(g)
            for db in range(NDB):
                ps = psum.tile([P, CH], F32, tag=f"b{6 + db % 2}")
                for cb in range(NCB):
                    nc.tensor.matmul(
                        ps[:],
                        wk_res[:, cb, db * P : (db + 1) * P],
                        xk[:, cb, :],
                        start=(cb == 0),
                        stop=(cb == NCB - 1),
                    )
                nc.vector.tensor_copy(kt_t[:, db, g * CH : (g + 1) * CH], ps[:])
            for i in range(4):
                kb = 4 * g + i
                for h in range(2):
                    ps = psum.tile([P, CH], F32, tag=f"b{(2 * i + h) % 4}")
                    for cb in range(NCB):
                        nc.tensor.matmul(
                            ps[:],
                            xk[:, cb, i * P : (i + 1) * P],
                            wv_res[:, cb, h * CH : (h + 1) * CH],
                            start=(cb == 0),
                            stop=(cb == NCB - 1),
                        )
                    nc.vector.tensor_copy(v_t[:, kb, h * CH : (h + 1) * CH], ps[:])

        # ---------------- Phase C: attention (software-pipelined) -------------
        qt_tiles = {}

        def qt_dma(c):
            k = c // 2
            s = SLOT_OF_PAIR[k]
            t = qtp.tile([P, NDB, QC], BF16, tag="qt", name=f"qt{c}")
            col = (c % 2) * QC
            nc.gpsimd.dma_start(
                t[:], qall_r[s][:, (k % 2) * NDB : (k % 2 + 1) * NDB, col : col + QC]
            )
            qt_tiles[c] = t

        for c in ORDER[:3]:
            qt_dma(c)

        prev = None  # (acc dict, j) pending drain
        for oi, j in enumerate(ORDER):
            if oi + 3 < NQC:
                qt_dma(ORDER[oi + 3])
            qt = qt_tiles.pop(j)
            acc = {}
            for sub in range(2):
                for c in range(3):
                    shape = [P, 2] if c == 2 else [P, 512]
                    acc[sub, c] = psum.tile(
                        shape, F32, tag=f"b{sub * 3 + c}", name=f"acc{j}_{sub}_{c}"
                    )

            def av(u, pt_t, first, last):
                for sub in range(2):
                    lhs = pt_t[:, sub * P : (sub + 1) * P]
                    nc.tensor.matmul(
                        acc[sub, 0][:], lhs, v_t[:, u, 0:512],
                        start=first, stop=last, skip_group_check=True,
                    )
                    nc.tensor.matmul(
                        acc[sub, 1][:], lhs, v_t[:, u, 512:1024],
                        start=first, stop=last, skip_group_check=True,
                    )
                    nc.tensor.matmul(
                        acc[sub, 2][:], lhs, v_t[:, u, D : D + 2],
                        start=first, stop=last, skip_group_check=True,
                    )

            def drain(d_acc, d_j):
                # All big copies on vector, in the order the next chunk's AV
                # matmuls will reuse the banks (b0, b3, b1, b4); exps stay
                # alone on scalar since they gate the AV weight loads. Row
                # sums land in the resident rs_all tile (no DMA until the
                # very end).
                ot = {}
                for sub in range(2):
                    ot[sub] = stg.tile([P, D], BF16, tag="stage", name=f"ot_{d_j}_{sub}")
                for c, sub in [(0, 0), (0, 1), (1, 0), (1, 1)]:
                    nc.vector.tensor_copy(
                        ot[sub][:, c * 512 : (c + 1) * 512], d_acc[sub, c][:]
                    )
                for sub in range(2):
                    row = d_j * QC + sub * P
                    col = 2 * d_j + sub
                    nc.scalar.copy(rs_all[:, col : col + 1], d_acc[sub, 2][:, 0:1])
                    nc.sync.dma_start(o_d[row : row + P, :], ot[sub][:])

            if prev is not None:  # drain at chunk start: copies overlap scores
                drain(*prev)
                prev = None

            pts = {}
            navs = [0]

            def do_av(u, last):
                av(u, pts.pop(u), first=(navs[0] == 0), last=last)
                navs[0] += 1

            for u in range(j + 1):
                st = psum.tile([P, QC], F32, tag=f"b{6 + u % 2}", name=f"st{j}_{u}")
                for db in range(NDB):
                    nc.tensor.matmul(
                        st[:],
                        kt_t[:, db, u * P : (u + 1) * P],
                        qt[:, db, :],
                        start=(db == 0),
                        stop=(db == NDB - 1),
                    )
                if u == j:
                    nc.vector.tensor_add(st[:], st[:], mask_t[:])
                pt = pp.tile([P, QC], BF16, tag="pt", name=f"pt{j}_{u}")
                nc.scalar.activation(pt[:], st[:], EXP, scale=EXPSCALE)
                pts[u] = pt
                if u >= 3:  # AV lag 3: drain copies + exps get a 3-block lead
                    do_av(u - 3, last=False)
            for u in range(max(0, j - 2), j + 1):
                do_av(u, last=(u == j))
            prev = (acc, j)
        drain_acc, drain_j = prev
        for sub in range(2):
            row = drain_j * QC + sub * P
            col = 2 * drain_j + sub
            ot = stg.tile([P, D], BF16, tag="stage", name=f"fot_{sub}")
            nc.vector.tensor_copy(ot[:, 0:512], drain_acc[sub, 0][:])
            nc.scalar.copy(ot[:, 512:1024], drain_acc[sub, 1][:])
            nc.scalar.copy(rs_all[:, col : col + 1], drain_acc[sub, 2][:, 0:1])
            nc.sync.dma_start(o_d[row : row + P, :], ot[:])
        nc.scalar.dma_start(rs_d[:], rs_all[:])

    nc.finalize()
    return nc


def _get_program():
    global _CACHED_NC
    if _CACHED_NC is None:
        _CACHED_NC = _build_program()
    return _CACHED_NC


def _masks():
    neg = np.float32(-1e30)
    tri = np.where(np.triu(np.ones((P, P), dtype=bool)), np.float32(0), neg)
    keep = np.zeros((P, P), dtype=np.float32)
    drop = np.full((P, P), neg, dtype=np.float32)
    return (
        np.ascontiguousarray(np.concatenate([tri, keep], axis=1)),  # even core
        np.ascontiguousarray(np.concatenate([drop, tri], axis=1)),  # odd core
    )


def kernel(x, Wq, Wk, Wv):
    out, _ = _run(x, Wq, Wk, Wv, trace=False)
    return out


def _run(x, Wq, Wk, Wv, trace=False, keep_res=False):
    bf = ml_dtypes.bfloat16
    x = np.asarray(x, dtype=np.float32)
    WqT = np.ascontiguousarray(np.asarray(Wq, dtype=np.float32).T.astype(bf))
    WkT = np.ascontiguousarray(np.asarray(Wk, dtype=np.float32).T.astype(bf))
    WvT = np.ascontiguousarray(np.asarray(Wv, dtype=np.float32).T.astype(bf))
    m_even, m_odd = _masks()
    ones2 = np.ascontiguousarray(
        np.tile(np.array([[1.0, 0.0] * NKB], dtype=np.float32), (P, 1)).astype(bf)
    )

    nc = _get_program()
    in_maps = []
    for core in range(8):
        b, p = core // 2, core % 2
        xT = np.ascontiguousarray(x[b].T.astype(bf))  # [D, T]
        xTk = np.ascontiguousarray(
            xT.reshape(D, T // P, P)[:, p::2, :].reshape(D, T // 2)
        )
        xqo = np.ascontiguousarray(
            np.concatenate(
                [
                    xT[:, CH * (k + p) : CH * (k + p + 1)]
                    for k in SLOT_PAIRS
                ],
                axis=1,
            )
        )
        in_maps.append(
            {
                "xqo": xqo,
                "xTk": xTk,
                "WqT": WqT,
                "WkT": WkT,
                "WvT": WvT,
                "mask": m_even if p == 0 else m_odd,
                "ones2": ones2,
            }
        )

    res = run_bass_kernel_spmd(nc, in_maps, core_ids=list(range(8)), trace=trace)
    if keep_res:
        global _LAST_RES
        _LAST_RES = res
    out = np.empty((B, T, D), dtype=np.float32)
    for b in range(B):
        O0 = res.results[2 * b]["O"].astype(np.float32)
        O1 = res.results[2 * b + 1]["O"].astype(np.float32)
        # rs[p, c] holds the row-sum for output row c*128 + p
        rs0 = res.results[2 * b]["rs"].T.reshape(T, 1)
        rs1 = res.results[2 * b + 1]["rs"].T.reshape(T, 1)
        out[b] = (O0 + O1) / (rs0 + rs1)
    return out, res.exec_time_ns
